# revision 1
# baseline (speedup 1.0000x reference)
"""Causal self-attention (B=4, T=2048, C=1024, H=16) on 8 TRN2 NeuronCores.

Sharding: (batch b, head-group g) -> core 2*b+g. Each core computes, for its
batch and its 8 heads: qkv projection, causal attention, and a partial output
projection restricted to its heads' feature columns. Host sums the two
head-group partials per batch and adds the projection bias plus the folded
v-bias term (bv_g @ Wp_g.T, softmax-invariant).

All matmul data is f16 (1 cycle/row on PE, half the DMA bytes of f32).

Device layouts (per core):
  xT    [1024, 2048] f16  x[b].T                 (contraction c on partitions)
  wqk   [1024, 1024] f16  [Wq_g | Wk_g].T
  wv    [1024, 512]  f16  Wv_g.T
  wp    [512, 1024]  f16  W_proj[:, cols_g].T
  q/k are produced feature-major ([d, t]) so scores S^T = kT.T @ qT come out
  [tk, tq]; v is token-major with a ones column per head so each AV matmul
  also accumulates the softmax denominator.
  AV runs in the cheap orientation: psav[q, 65] += E_block.T @ V_aug — the
  matmul free dim is 65 (vs 512 queries), saving ~80k PE rows per core.
  The per-(query, head) normalization is a DVE multiply with a stride-0
  broadcast reciprocal; y ([q, f], query-major) is then transposed to
  feature-major yT via DMA-XBAR transposes for the output projection.
  Emission interleaves proj(q+1) and outproj(q-1) into attention(q) so the
  in-order PE queue stays fed while ACT streams the exps.
"""

import numpy as np

N_CORES = 8
B, T, C, H, D = 4, 2048, 1024, 16, 64
F = 512          # features per head-group (8 heads x 64)
TQ = 512         # query block (matmul free dim)
TK = 128         # key block (psum partition dim)

_CACHE = {}


def _build_bass(debug=False):
    import sys
    if '/opt/trn_rl_repo' not in sys.path:
        sys.path.insert(0, '/opt/trn_rl_repo')
    import concourse.tile as tile
    from concourse import bacc, mybir

    f32 = mybir.dt.float32
    f16 = mybir.dt.float16
    AF = mybir.ActivationFunctionType

    nc = bacc.Bacc("TRN2", target_bir_lowering=False, debug=False,
                   num_devices=N_CORES)
    xT = nc.dram_tensor("xT", [C, T], f16, kind="ExternalInput").ap()
    wqk = nc.dram_tensor("wqk", [C, 2 * F], f16, kind="ExternalInput").ap()
    wv = nc.dram_tensor("wv", [C, F], f16, kind="ExternalInput").ap()
    wp = nc.dram_tensor("wp", [F, C], f16, kind="ExternalInput").ap()
    bqk = nc.dram_tensor("bqk", [128, 8], f32, kind="ExternalInput").ap()
    ident = nc.dram_tensor("ident", [128, 128], f16, kind="ExternalInput").ap()
    masks = nc.dram_tensor("masks", [TK, 4 * TQ], f16,
                           kind="ExternalInput").ap()
    part = nc.dram_tensor("part", [T, C], f16, kind="ExternalOutput").ap()
    if debug:
        d_qT = nc.dram_tensor("d_qT", [128, 4, T], f16, kind="ExternalOutput").ap()
        d_kT = nc.dram_tensor("d_kT", [128, 4, T], f16, kind="ExternalOutput").ap()
        d_v = nc.dram_tensor("d_v", [128, 16, 8, D + 1], f16,
                             kind="ExternalOutput").ap()
        d_yT = nc.dram_tensor("d_yT", [128, 4, T], f16, kind="ExternalOutput").ap()
        d_ya = nc.dram_tensor("d_ya", [128, 4, F], f16, kind="ExternalOutput").ap()
        d_E = nc.dram_tensor("d_E", [128, 1024], f16, kind="ExternalOutput").ap()

    with tile.TileContext(nc) as tc:
        with (tc.tile_pool(name="singles", bufs=1) as S,
              tc.tile_pool(name="xq", bufs=4) as XQ,
              tc.tile_pool(name="ep", bufs=6) as EP,
              tc.tile_pool(name="yb", bufs=2) as YB,
              tc.tile_pool(name="rc", bufs=4) as RC,
              tc.tile_pool(name="ob", bufs=3) as OB,
              tc.tile_pool(name="psqk", bufs=2, space="PSUM") as PSQK,
              tc.tile_pool(name="psav", bufs=2, space="PSUM") as PSAV,
              tc.tile_pool(name="psbp", bufs=2, space="PSUM") as PSBP):
            wqk_sb = S.tile([128, 8, 2 * F], f16, tag="wqk")
            wv_sb = S.tile([128, 8, F], f16, tag="wv")
            wp_sb = S.tile([128, 4, C], f16, tag="wp")
            bqk_sb = S.tile([128, 8], f32, tag="bqk")
            mask_sb = S.tile([128, 4 * TQ], f16, tag="masks")
            qT = S.tile([128, 4, T], f16, tag="qT")
            kT = S.tile([128, 4, T], f16, tag="kT")
            v_aug = S.tile([128, 16, 8, D + 1], f16, tag="v_aug")
            yT = S.tile([128, 4, T], f16, tag="yT")

            # Startup DMAs, fine-grained and interleaved so the first proj
            # matmul can start after ~2 transfers and supply stays ahead of
            # the PE: wqk in [512c, 256r] chunks alternating with x halves.
            def wqk_chunk(ch, j):
                nc.sync.dma_start(
                    out=wqk_sb[:, 4 * ch:4 * ch + 4, 256 * j:256 * j + 256],
                    in_=wqk[512 * ch:512 * ch + 512, 256 * j:256 * j + 256]
                    .rearrange("(k p) r -> p k r", p=128))

            xq0 = [XQ.tile([128, 4, TQ], f16, tag="xq", name="xq0a"),
                   XQ.tile([128, 4, TQ], f16, tag="xq", name="xq0b")]

            def xq0_chunk(ch, th):
                nc.sync.dma_start(
                    out=xq0[ch][:, :, 256 * th:256 * th + 256],
                    in_=xT[512 * ch:512 * ch + 512, 256 * th:256 * th + 256]
                    .rearrange("(k p) t -> p k t", p=128))

            # both c-halves of the t0 quarter first: the r0 group's ck4-7
            # matmuls unblock ~1us sooner
            wqk_chunk(0, 0)
            xq0_chunk(0, 0)
            wqk_chunk(1, 0)
            xq0_chunk(1, 0)
            nc.sync.dma_start(out=bqk_sb, in_=bqk)
            xq0_chunk(0, 1)
            xq0_chunk(1, 1)
            for ch in range(2):
                nc.sync.dma_start(
                    out=wv_sb[:, 4 * ch:4 * ch + 4, :],
                    in_=wv[512 * ch:512 * ch + 512, :]
                    .rearrange("(k p) r -> p k r", p=128))
            for j in range(1, 4):
                wqk_chunk(0, j)
                wqk_chunk(1, j)
            ident_sb = S.tile([128, 128], f16, tag="ident")

            # PE ramp warmup: the cost model runs the PE at 0.65/1.2 GHz for
            # the first ~3us of a busy stretch. Dependency-free dummy
            # matmuls on scratch SBUF burn the ramp while the first input
            # DMAs are still in flight, so real matmuls start at 2.4 GHz.
            ps_w = PSBP.tile([128, TQ], f32, tag="bp")
            for i in range(8):
                nc.tensor.matmul(ps_w, mask_sb[:, 0:128], mask_sb[:, 0:512],
                                 start=(i == 0), stop=(i == 7))
            nc.vector.memset(v_aug[:, :, :, D:D + 1], 1.0)

            xqh = {0: xq0}

            def emit_xq_dma(qq):
                t0 = TQ * qq
                tiles = []
                for ch in range(2):
                    xq = XQ.tile([128, 4, TQ], f16, tag="xq")
                    nc.sync.dma_start(
                        out=xq,
                        in_=xT[512 * ch:512 * ch + 512, t0:t0 + TQ]
                        .rearrange("(k p) t -> p k t", p=128))
                    tiles.append(xq)
                xqh[qq] = tiles

            def emit_proj_group(qq, unit, tsplit=False):
                """unit 0..7 = q/k r-blocks, 8..11 = v token-blocks."""
                t0 = TQ * qq
                xq = xqh[qq]
                if unit < 8:
                    r = unit
                    ps = PSBP.tile([128, TQ], f32, tag="bp")
                    # tsplit: two accumulation groups over t-halves so the
                    # first matmuls start as soon as the first quarter-size
                    # x chunk lands (startup only)
                    for th in ((0, 1) if tsplit else (0,)):
                        w_t = 256 if tsplit else TQ
                        for ck in range(8):
                            nc.tensor.matmul(
                                ps[:, w_t * th:w_t * th + w_t],
                                wqk_sb[:, ck, 128 * r:128 * r + 128],
                                xq[ck // 4][:, ck % 4, w_t * th:w_t * th + w_t],
                                start=(ck == 0), stop=(ck == 7))
                    dest = qT if r < 4 else kT
                    nc.vector.tensor_scalar_add(
                        dest[:, r % 4, t0:t0 + TQ], ps, bqk_sb[:, r:r + 1])
                else:
                    tt = unit - 8
                    vt = 4 * qq + tt
                    psv = PSBP.tile([128, TQ], f32, tag="bp")
                    for ck in range(8):
                        nc.tensor.matmul(psv,
                                         xq[ck // 4][:, ck % 4,
                                                     128 * tt:128 * tt + 128],
                                         wv_sb[:, ck, :],
                                         start=(ck == 0), stop=(ck == 7))
                    nc.vector.tensor_copy(
                        out=v_aug[:, vt, :, 0:D],
                        in_=psv.rearrange("p (h d) -> p h d", h=8))

            def emit_outproj_tt(qq, tt, tail=False):
                t = TQ * qq + 128 * tt
                outsb = OB.tile([128, 2, TQ], f16, tag="ob")
                for jh in range(2):
                    if tail and jh == 1:
                        qk2 = PSQK.tile([128, 1024], f32, tag="qk")
                        pso = qk2[:, 0:512]
                    else:
                        pso = PSBP.tile([128, TQ], f32, tag="bp")
                    for ft in range(4):
                        nc.tensor.matmul(pso, yT[:, ft, t:t + 128],
                                         wp_sb[:, ft, 512 * jh:512 * jh + 512],
                                         start=(ft == 0), stop=(ft == 3))
                    if tail and jh == 1:
                        # post-exp epilogue: ACT is idle, split the copy work
                        nc.scalar.activation(out=outsb[:, jh, :], in_=pso,
                                             func=AF.Identity)
                    else:
                        nc.vector.tensor_copy(out=outsb[:, jh, :], in_=pso)
                    if tail:
                        nc.sync.dma_start(
                            out=part[t:t + 128, 512 * jh:512 * jh + 512],
                            in_=outsb[:, jh, :])
                if not tail:
                    nc.sync.dma_start(out=part[t:t + 128, :],
                                      in_=outsb.rearrange("p a b -> p (a b)"))

            def emit_attn_head(qq, h, units=(), slots=()):
                """Scores + exp + AV (cheap orientation) + normalize for one
                head of query quarter qq. Software-pipelined: AV(kp-1) is
                emitted after scores(kp) so PE never waits on exp(kp).
                `units` are interleave closures emitted at kp in `slots`
                (between scores(kp) and AV(kp-1)) — filler PE work that lets
                ACT catch up on exps in the ACT-heavy late quarters."""
                t0 = TQ * qq
                hp, par = h // 2, h % 2
                n_tkb = 4 * qq + 4
                units = list(units)
                psav = PSAV.tile([128, 4, D + 1], f32, tag="av")
                ews = []

                def emit_av(kp):
                    Ew = ews[kp]
                    for half in range(2):
                        tkb = 2 * kp + half
                        for q2 in range(4):
                            if tkb <= 4 * qq + q2:
                                # start=True clears has_written for the WHOLE
                                # bank, so only the very first write may set
                                # it; the other q2 slices' first writes
                                # overwrite because that clear unset their
                                # bits (has_written=0 -> overwrite).
                                nc.tensor.matmul(
                                    psav[:, q2, :],
                                    Ew[:, 512 * half + 128 * q2:
                                       512 * half + 128 * q2 + 128],
                                    v_aug[:, tkb, h, :],
                                    start=(tkb == 0 and q2 == 0),
                                    stop=(tkb == 4 * qq + q2))

                for kp in range(n_tkb // 2):
                    ps2 = PSQK.tile([128, 1024], f32, tag="qk")
                    for half in range(2):
                        tkb = 2 * kp + half
                        d = tkb - 4 * qq
                        c0 = 128 * d if d > 0 else 0
                        nc.tensor.matmul(
                            ps2[:, 512 * half + c0:512 * half + 512],
                            kT[64 * par:64 * par + 64, hp,
                               TK * tkb:TK * tkb + TK],
                            qT[64 * par:64 * par + 64, hp,
                               t0 + c0:t0 + TQ],
                            start=True, stop=True)
                    Ew = EP.tile([128, 1024], f16, tag="E")
                    ews.append(Ew)
                    d0 = 2 * kp - 4 * qq
                    e0 = 128 * d0 if d0 > 0 else 0
                    nc.scalar.activation(out=Ew[:, e0:], in_=ps2[:, e0:],
                                         func=AF.Exp, scale=0.125)
                    if d0 >= 0:
                        nc.vector.tensor_mul(
                            out=Ew[:, e0:], in0=Ew[:, e0:],
                            in1=mask_sb[:, 512 * d0 + e0:512 * d0 + 1024])
                    if debug and qq == 0 and h == 0 and kp == 0:
                        nc.sync.dma_start(out=d_E, in_=Ew)
                    if kp in slots and units:
                        units.pop(0)()
                    # AV lags the exp stream by TWO pairs so it never waits
                    # on an in-flight exp
                    if kp > 1:
                        emit_av(kp - 2)
                # leftover units BEFORE the diagonal AV flush: deferred
                # v-units must precede the diag AVs that read them
                for u in units:
                    u()
                if n_tkb // 2 > 1:
                    emit_av(n_tkb // 2 - 2)
                emit_av(n_tkb // 2 - 1)

                rcp = RC.tile([128, 4], f32, tag="rcp")
                nc.vector.reciprocal(
                    out=rcp,
                    in_=psav[:, :, D:D + 1].rearrange("p a o -> p (a o)"))
                return psav, rcp

            def emit_norm_transpose(qq, h, psav, rcp, y_all):
                t0 = TQ * qq
                hp = h // 2
                nc.vector.tensor_mul(
                    out=y_all[:, :, D * h:D * h + D],
                    in0=psav[:, :, 0:D],
                    in1=rcp.unsqueeze(2).broadcast_to((128, 4, D)))
                if h % 2 != 1:
                    return
                if qq == 3 and h == 7:
                    # Last head-pair of the kernel: the DMA-XBAR transpose
                    # latency (~3us HWDGE chain) would gate outproj(3), so
                    # run it on the idle PE + DVE instead.
                    # same tag as psav: reuses the (freed) h6 slot, and the
                    # bank-granular slot already fits [128, 4, 128] f32
                    tp = PSAV.tile([128, 4, 128], f16, tag="av")
                    for q2 in range(4):
                        nc.tensor.transpose(
                            tp[:, q2, :],
                            y_all[:, q2, 128 * hp:128 * hp + 128], ident_sb)
                    for q2 in range(4):
                        # post-exp epilogue: ACT is idle, keep DVE clear for
                        # the outproj copies
                        nc.scalar.activation(
                            out=yT[:, hp, t0 + 128 * q2:t0 + 128 * q2 + 128],
                            in_=tp[:, q2, :], func=AF.Identity)
                    return
                for q2 in range(4):
                    nc.sync.dma_start(
                        out=yT[:, hp, t0 + 128 * q2:t0 + 128 * q2 + 128],
                        in_=y_all[:, q2, 128 * hp:128 * hp + 128],
                        transpose=True)

            # ---- main schedule ----
            # proj(0) in DMA-supply order: r0,r1 (first wqk chunks, t-split
            # to track the quarter-size x chunks), v (wv lands next), then
            # r2..r7
            for unit in (0, 1, 8, 9, 10, 11, 2, 3, 4, 5, 6, 7):
                emit_proj_group(0, unit, tsplit=unit in (0, 1))
            nc.sync.dma_start(out=mask_sb, in_=masks)
            nc.sync.dma_start(out=wp_sb,
                              in_=wp.rearrange("(k p) r -> p k r", p=128))
            emit_xq_dma(1)
            nc.sync.dma_start(out=ident_sb, in_=ident)

            # Per-quarter interleave work queues (closures). Late quarters
            # are ACT-bound, so PE filler is parked there: proj(3) and
            # outproj(0) inside attention(2); outproj(1,2) inside
            # attention(3) — emitted at in-loop slots mid-head.
            def U_proj(qq, u):
                return lambda: emit_proj_group(qq, u)

            def U_out(qq, tt):
                return lambda: emit_outproj_tt(qq, tt)

            outsb_half = {}

            def U_out_jh(qq, tt, jh):
                def emit():
                    t = TQ * qq + 128 * tt
                    if jh == 0:
                        outsb_half[(qq, tt)] = OB.tile([128, 2, TQ], f16,
                                                       tag="ob", name="osb")
                    outsb = outsb_half[(qq, tt)]
                    pso = PSBP.tile([128, TQ], f32, tag="bp")
                    for ft in range(4):
                        nc.tensor.matmul(pso, yT[:, ft, t:t + 128],
                                         wp_sb[:, ft, 512 * jh:512 * jh + 512],
                                         start=(ft == 0), stop=(ft == 3))
                    nc.vector.tensor_copy(out=outsb[:, jh, :], in_=pso)
                    if jh == 1:
                        nc.sync.dma_start(
                            out=part[t:t + 128, :],
                            in_=outsb.rearrange("p a b -> p (a b)"))
                return emit

            def U_xq(qq):
                return lambda: emit_xq_dma(qq)

            # Partial outproj(3) groups: ft{0,1,2} emitted inside h7's kp
            # loop (their yT chunks are ready after h5); the hp=3-dependent
            # ft3 lands in the epilogue. h7 gets ONLY these as units — any
            # other bp-pool consumer there would deadlock the ring against
            # the held partial psum tiles.
            partials = {}

            def U_partial(tt):
                def emit():
                    ps = PSBP.tile([128, TQ], f32, tag="bp")
                    t = TQ * 3 + 128 * tt
                    for ft in range(3):
                        nc.tensor.matmul(ps, yT[:, ft, t:t + 128],
                                         wp_sb[:, ft, 0:512],
                                         start=(ft == 0), stop=False)
                    partials[tt] = ps
                return emit

            # proj(q)'s v-groups are only consumed by attention(q)'s diagonal
            # pairs (the tail of each head), so v(2) can slide into window 2
            # — net-new PE filler for the ACT-bound late windows.
            work = {
                0: [U_xq(2)] + [U_proj(1, u) for u in range(12)],
                1: [U_xq(3)] + [U_proj(2, u) for u in range(8)],
                2: [U_proj(2, u) for u in range(8, 12)]
                   + [U_proj(3, u) for u in range(8)],
                3: [U_proj(3, u) for u in range(8, 12)]
                   + [U_out_jh(qx, tt, jh) for qx in range(3)
                      for tt in range(4) for jh in range(2)],
            }
            slots = {0: (), 1: (), 2: (1, 2, 3, 4), 3: (1, 2, 3, 4, 5, 7)}

            for qq in range(4):
                w = work[qq]
                nw = len(w)
                y_all = YB.tile([128, 4, F], f16, tag="y_all")
                def esplit(n, k):
                    return [((i + 1) * n) // k - (i * n) // k
                            for i in range(k)]

                # h0 of qq>=2 MUST emit all 4 deferred v-units in-loop: its
                # own diagonal AV (post-loop, lag-2 flush) reads all four
                # new v_aug blocks — a later-head unit would be a
                # program-order race (read-before-write). h7 of qq3 runs two
                # reserved jh-units early, then the partial outproj(3)
                # groups (which hold the bp ring, so they must be h7's LAST
                # bp consumers).
                if qq < 2:
                    counts = esplit(nw, 8)
                elif qq == 2:
                    counts = [4] + esplit(nw - 4, 7)
                else:
                    # ACT lag accumulates through the window: bias the
                    # filler toward the later heads
                    counts = [4, 2, 3, 3, 4, 5, 5, 2]
                for h in range(8):
                    take, w = w[:counts[h]], w[counts[h]:]
                    if qq == 3 and h == 7:
                        take = take + [U_partial(0), U_partial(1)]
                    if qq < 2:
                        # early quarters are PE-bound: emit after the head
                        psav, rcp = emit_attn_head(qq, h)
                        emit_norm_transpose(qq, h, psav, rcp, y_all)
                        for u in take:
                            u()
                    else:
                        psav, rcp = emit_attn_head(qq, h, units=take,
                                                   slots=slots[qq])
                        emit_norm_transpose(qq, h, psav, rcp, y_all)
                if debug and qq == 0:
                    nc.sync.dma_start(out=d_ya, in_=y_all)
            # epilogue: finish the two partial groups (jh=0 halves of tt=0,1)
            for tt in (0, 1):
                t = TQ * 3 + 128 * tt
                ps = partials[tt]
                nc.tensor.matmul(ps, yT[:, 3, t:t + 128], wp_sb[:, 3, 0:512],
                                 start=False, stop=True)
                outsb = OB.tile([128, 2, TQ], f16, tag="ob")
                nc.vector.tensor_copy(out=outsb[:, 0, :], in_=ps)
                nc.sync.dma_start(out=part[t:t + 128, 0:512],
                                  in_=outsb[:, 0, :])
                qk2 = PSQK.tile([128, 1024], f32, tag="qk")
                pso = qk2[:, 0:512]
                for ft in range(4):
                    nc.tensor.matmul(pso, yT[:, ft, t:t + 128],
                                     wp_sb[:, ft, 512:1024],
                                     start=(ft == 0), stop=(ft == 3))
                nc.scalar.activation(out=outsb[:, 1, :], in_=pso,
                                     func=AF.Identity)
                nc.sync.dma_start(out=part[t:t + 128, 512:1024],
                                  in_=outsb[:, 1, :])
            for tt in (2, 3):
                emit_outproj_tt(3, tt, tail=True)
            if debug:
                nc.sync.dma_start(out=d_qT, in_=qT)
                nc.sync.dma_start(out=d_kT, in_=kT)
                nc.sync.dma_start(out=d_v, in_=v_aug)
                nc.sync.dma_start(out=d_yT, in_=yT)

    nc.compile()
    return nc


def _get_nc():
    if "nc" not in _CACHE:
        _CACHE["nc"] = _build_bass()
    return _CACHE["nc"]


def _make_in_maps(x, W_attn, b_attn, W_proj):
    x = np.asarray(x, dtype=np.float32)
    W_attn = np.asarray(W_attn, dtype=np.float32)
    b_attn = np.asarray(b_attn, dtype=np.float32)
    W_proj = np.asarray(W_proj, dtype=np.float32)

    jj = np.arange(TQ)[None, :]
    ii = np.arange(TK)[:, None]
    # Staircase mask for the diagonal key-quad: column block hh (of 4)
    # holds key block (4*tqb + hh); valid iff local j >= 128*hh + i.
    masks = np.concatenate([(jj >= 128 * hh + ii) for hh in range(4)],
                           axis=1).astype(np.float16)

    in_maps = []
    for c in range(N_CORES):
        b, g = divmod(c, 2)
        wq = W_attn[F * g:F * g + F]
        wk = W_attn[C + F * g:C + F * g + F]
        wv_ = W_attn[2 * C + F * g:2 * C + F * g + F]
        bqk_flat = np.concatenate([b_attn[F * g:F * g + F],
                                   b_attn[C + F * g:C + F * g + F]])
        in_maps.append({
            "xT": np.ascontiguousarray(x[b].T).astype(np.float16),
            "wqk": np.ascontiguousarray(
                np.concatenate([wq, wk], axis=0).T).astype(np.float16),
            "wv": np.ascontiguousarray(wv_.T).astype(np.float16),
            "wp": np.ascontiguousarray(
                W_proj[:, F * g:F * g + F].T).astype(np.float16),
            "bqk": np.ascontiguousarray(bqk_flat.reshape(8, 128).T),
            "ident": np.eye(128, dtype=np.float16),
            "masks": masks,
        })
    return in_maps


def kernel(x, W_attn, b_attn, W_proj, b_proj):
    import sys
    if '/opt/trn_rl_repo' not in sys.path:
        sys.path.insert(0, '/opt/trn_rl_repo')
    from concourse.bass_utils import run_bass_kernel_spmd

    nc = _get_nc()
    in_maps = _make_in_maps(x, W_attn, b_attn, W_proj)
    res = run_bass_kernel_spmd(nc, in_maps, core_ids=list(range(N_CORES)))
    b_proj = np.asarray(b_proj, dtype=np.float32)
    W_proj = np.asarray(W_proj, dtype=np.float32)
    b_attn = np.asarray(b_attn, dtype=np.float32)
    # v-bias is softmax-invariant: its contribution is a constant row
    # bv_g @ Wp_g.T, folded host-side along with b_proj.
    const = b_proj.copy()
    for g in range(2):
        bv_g = b_attn[2 * C + F * g:2 * C + F * g + F]
        const += bv_g @ W_proj[:, F * g:F * g + F].T
    out = np.empty((B, T, C), dtype=np.float32)
    for b in range(B):
        out[b] = (res.results[2 * b]["part"].astype(np.float32)
                  + res.results[2 * b + 1]["part"].astype(np.float32)
                  + const[None, :])
    return out

